# revision 9
# baseline (speedup 1.0000x reference)
"""Trainium2 Bass kernel for nn_MultiHeadGATEAULayer (GNN message passing).

Strategy (8 NeuronCores, no collectives):
  - Host sorts edges by target node; nodes are split into 8 contiguous ranges
    (6250 nodes/core) so each core owns its targets' incoming edges.
  - Per-edge tensors (edge features via edge_map) are sharded to cores on the
    host; node tables are replicated for the device-side src gathers.
  - On device, per core: edges stream in 128-edge blocks grouped by 128-node
    windows.  Per block: gather X[src] (dma_gather, two int16-index chunks with
    an overlay trick), project gathered rows + edge-feature rows with fused
    [W | W' | W@aT] weights on the TensorEngine (float32r), add the per-target
    table via a one-hot expand matmul, softmax weights via Lrelu+Exp on ACT
    (no max-subtraction needed; scores are bounded), and scatter-add the
    weighted values into the node window with a one-hot matmul accumulated in
    PSUM.  Segment softmax normalization happens once per node at the end.
  - Outputs (new_final node shards + new_edge_feature edge shards) are
    assembled/unpermuted on the host.
"""

import sys

for _p in ("/opt/trn_rl_repo",):
    if _p not in sys.path:
        sys.path.insert(0, _p)

import numpy as np

import concourse.bass as bass
import concourse.bacc as bacc
import concourse.mybir as mybir
import concourse.tile as tile
from concourse.bass_utils import run_bass_kernel_spmd
from concourse.masks import make_identity

dt = mybir.dt
ACT = mybir.ActivationFunctionType
ALU = mybir.AluOpType

P = 128
H = 8
HD = 16
D = 128
NEG_SLOPE = 0.2
N_CORES = 8
CHUNK = 32768  # int16 index limit for dma_gather


def _pack_idx16(idx_lists, slotsw):
    """Pack per-window int16 index lists into the [128, W*slotsw//16] wrapped
    layout dma_gather expects: index i of window w lives at
    [16*k + i%16, w*slotsw//16 + i//16] for every k (replicated)."""
    Wn = len(idx_lists)
    sw16 = slotsw // 16
    out = np.zeros((P, Wn * sw16), np.int16)
    for w, lst in enumerate(idx_lists):
        a = np.asarray(lst, np.int16).reshape(sw16, 16)  # i = s*16 + p
        blk = a.T  # [16, sw16]
        out[:, w * sw16:(w + 1) * sw16] = np.tile(blk, (P // 16, 1))
    return out


def _host_prep(inputs):
    X = np.ascontiguousarray(inputs["node_feature_matrix"], np.float32)
    EF = np.ascontiguousarray(inputs["edge_feature_matrix"], np.float32)
    G = np.ascontiguousarray(inputs["global_node_features"], np.float32)
    tgt = np.asarray(inputs["edge_index"][0]).astype(np.int64)
    src = np.asarray(inputs["edge_index"][1]).astype(np.int64)
    em = np.asarray(inputs["edge_map"]).astype(np.int64)
    N = X.shape[0]
    E = tgt.shape[0]
    NPC = N // N_CORES
    W = (NPC + P - 1) // P

    order = np.argsort(tgt, kind="stable")
    tgt_s, src_s, em_s = tgt[order], src[order], em[order]

    # per-core windows: core c, window w covers nodes [c*NPC + w*128, ...)
    # (core boundaries are not 128-aligned, so windows must be per-core)
    bounds = []
    for c in range(N_CORES):
        n0 = c * NPC
        bounds.extend(n0 + w * P for w in range(W))
    bounds.append(N)
    win_starts_all = np.searchsorted(tgt_s, np.asarray(bounds))
    wcnt = np.diff(win_starts_all)
    B = int(np.ceil(wcnt.max() / P))
    slotsw = B * P

    aT = np.asarray(inputs["a_proj_w"], np.float32).T  # [128, 8]
    Wv = np.asarray(inputs["Wv"], np.float32)
    Wh = np.asarray(inputs["Wh"], np.float32)
    We = np.asarray(inputs["We"], np.float32)
    Wg = np.asarray(inputs["Wg"], np.float32)
    Wu = np.asarray(inputs["Wu"], np.float32)
    Wge = np.asarray(inputs["W_global_edge"], np.float32)

    wcomb_x = np.concatenate([Wv, Wh, Wv @ aT], axis=1)  # [128, 264]
    wcomb_e = np.concatenate([We, Wg, We @ aT], axis=1)
    wu_a = np.zeros((D, 264), np.float32)
    wu_a[:, 0:128] = Wu
    wu_a[:, 256:264] = Wu @ aT
    wge_a = np.zeros((D, 264), np.float32)
    wge_a[:, 0:128] = Wge
    wge_a[:, 256:264] = Wge @ aT

    common = {
        "x_lo": X[:CHUNK],
        "x_hi": np.ascontiguousarray(X[CHUNK:]),
        "wcomb_x": wcomb_x,
        "wcomb_e": wcomb_e,
        "wu_a": wu_a,
        "wge_a": wge_a,
        "w0": np.asarray(inputs["W0"], np.float32),
        "wgn": np.asarray(inputs["W_global_node"], np.float32),
        "wout_t": np.ascontiguousarray(np.asarray(inputs["W_out_w"], np.float32).T),
        "abias": np.broadcast_to(np.asarray(inputs["a_proj_b"], np.float32), (P, H)).copy(),
        "obias": np.broadcast_to(np.asarray(inputs["W_out_b"], np.float32), (P, D)).copy(),
        "iota": np.broadcast_to(np.arange(P, dtype=np.float32), (P, P)).copy(),
    }

    per_core = []
    assemble = []
    for c in range(N_CORES):
        n0 = c * NPC
        idx_a_lists = []
        idx_b_lists = []
        cnt_b = np.zeros(W, np.int32)
        tgt_rel = np.full((W * B * P,), -1.0, np.float32)
        slot_orig = np.full((W * B * P,), -1, np.int64)
        slot_src = np.zeros((W * B * P,), np.int64)
        slot_em = np.zeros((W * B * P,), np.int64)
        for w in range(W):
            gw = c * W + w
            e_lo, e_hi = win_starts_all[gw], win_starts_all[gw + 1]
            ew = np.arange(e_lo, e_hi)
            is_b = src_s[ew] >= CHUNK
            ew = np.concatenate([ew[is_b], ew[~is_b]])  # B-chunk edges first
            nb = int(is_b.sum())
            na = len(ew) - nb
            if nb == 0:
                # overlay call 1 would have an empty index list; give it one
                # dummy row and shift real edges right by one pad slot.
                assert len(ew) < slotsw, "window exactly full with no B edges"
                base = 1
                nb_eff = 1
                idx_b = np.full(slotsw, -1, np.int16)
                idx_b[0] = 0
            else:
                base = 0
                nb_eff = nb
                idx_b = np.full(slotsw, -1, np.int16)
                idx_b[:nb] = (src_s[ew[:nb]] - CHUNK).astype(np.int16)
            assert base + len(ew) <= slotsw, "window overflow"
            idx_a = np.zeros(slotsw, np.int16)
            idx_a[base + nb:base + nb + na] = src_s[ew[nb:]].astype(np.int16)
            idx_a_lists.append(idx_a)
            idx_b_lists.append(idx_b)
            assert nb_eff <= 1024, 'B-chunk prefix exceeds one gather sub-call'
            cnt_b[w] = nb_eff
            sl = w * slotsw + base
            tgt_rel[sl:sl + len(ew)] = (tgt_s[ew] - (n0 + w * P)).astype(np.float32)
            slot_orig[sl:sl + len(ew)] = order[ew]
            slot_src[sl:sl + len(ew)] = src_s[ew]
            slot_em[sl:sl + len(ew)] = em_s[ew]

        # feature-major edge-feature blocks: [W*B, 128f, 128e]
        ef_rows = EF[slot_em]  # [W*B*P, 128]
        eft = np.ascontiguousarray(
            ef_rows.reshape(W * B, P, D).transpose(0, 2, 1)
        ).reshape(W * B * P, D)

        xc = np.zeros((W * P, D), np.float32)
        xc[:NPC] = X[n0:n0 + NPC]
        gc = np.zeros((W * P, D), np.float32)
        gc[:NPC] = G[n0:n0 + NPC]

        # tgt_rel in partition-major layout [p, w*B + b]
        tr = tgt_rel.reshape(W * B, P).T.copy()  # [128, W*B]

        per_core.append({
            "xc": xc,
            "gc": gc,
            "eft": eft,
            "idx_a": _pack_idx16(idx_a_lists, slotsw),
            "idx_b": _pack_idx16(idx_b_lists, slotsw),
            "cnt_b": cnt_b.reshape(1, W),
            "tgt_rel": tr,
            **common,
        })
        assemble.append({"slot_orig": slot_orig, "n0": n0})

    meta = {
        "N": N, "E": E, "NPC": NPC, "W": W, "B": B, "slotsw": slotsw,
        "n_hi": N - CHUNK,
    }
    return per_core, assemble, meta


def _build_nc(meta):
    W, B, slotsw = meta["W"], meta["B"], meta["slotsw"]
    n_hi = meta["n_hi"]
    sw16 = slotsw // 16
    f32, f32r, i16, i32 = dt.float32, dt.float32r, dt.int16, dt.int32

    nc = bacc.Bacc(num_swdge_queues=4)
    X_LO = nc.dram_tensor("x_lo", [CHUNK, D], f32r, kind="ExternalInput")
    X_HI = nc.dram_tensor("x_hi", [n_hi, D], f32r, kind="ExternalInput")
    XC = nc.dram_tensor("xc", [W * P, D], f32r, kind="ExternalInput")
    GC = nc.dram_tensor("gc", [W * P, D], f32r, kind="ExternalInput")
    EFT = nc.dram_tensor("eft", [W * B * P, D], f32r, kind="ExternalInput")
    IDX_A = nc.dram_tensor("idx_a", [P, W * sw16], i16, kind="ExternalInput")
    IDX_B = nc.dram_tensor("idx_b", [P, W * sw16], i16, kind="ExternalInput")
    CNT_B = nc.dram_tensor("cnt_b", [1, W], i32, kind="ExternalInput")
    TGT_REL = nc.dram_tensor("tgt_rel", [P, W * B], f32, kind="ExternalInput")
    WCX = nc.dram_tensor("wcomb_x", [D, 264], f32r, kind="ExternalInput")
    WCE = nc.dram_tensor("wcomb_e", [D, 264], f32r, kind="ExternalInput")
    WUA = nc.dram_tensor("wu_a", [D, 264], f32r, kind="ExternalInput")
    WGA = nc.dram_tensor("wge_a", [D, 264], f32r, kind="ExternalInput")
    W0 = nc.dram_tensor("w0", [D, D], f32r, kind="ExternalInput")
    WGN = nc.dram_tensor("wgn", [D, D], f32r, kind="ExternalInput")
    WOT = nc.dram_tensor("wout_t", [D, D], f32r, kind="ExternalInput")
    ABIAS = nc.dram_tensor("abias", [P, H], f32, kind="ExternalInput")
    OBIAS = nc.dram_tensor("obias", [P, D], f32, kind="ExternalInput")
    IOTA = nc.dram_tensor("iota", [P, P], f32, kind="ExternalInput")
    NEF = nc.dram_tensor("nef", [W * B * P, D], f32, kind="ExternalOutput")
    NEWF = nc.dram_tensor("newf", [W * P, D], f32, kind="ExternalOutput")
    AGGD = nc.dram_tensor("aggd", [P, W * 136], f32, kind="ExternalOutput")

    with tile.TileContext(nc) as tc:
        with (
            tc.tile_pool(name="persist", bufs=1) as pp,
            tc.tile_pool(name="gx", bufs=3) as gxp,
            tc.tile_pool(name="ef", bufs=4) as efp,
            tc.tile_pool(name="work", bufs=4) as wp,
            tc.tile_pool(name="small", bufs=4) as sp,
            tc.tile_pool(name="out", bufs=4) as op,
            tc.tile_pool(name="pt", bufs=4, space="PSUM") as ptp,
            tc.tile_pool(name="pmain", bufs=2, space="PSUM") as pmp,
            tc.tile_pool(name="pagg", bufs=2, space="PSUM") as pap,
        ):
            ident_f = pp.tile([P, P], f32)
            make_identity(nc, ident_f[:])
            ident = pp.tile([P, P], f32r)
            nc.vector.tensor_copy(ident[:], ident_f[:])
            iota_t = pp.tile([P, P], f32)
            nc.sync.dma_start(iota_t[:], IOTA[:])
            wcx_t = pp.tile([D, 264], f32r)
            nc.sync.dma_start(wcx_t[:], WCX[:])
            wce_t = pp.tile([D, 264], f32r)
            nc.sync.dma_start(wce_t[:], WCE[:])
            wua_t = pp.tile([D, 264], f32r)
            nc.sync.dma_start(wua_t[:], WUA[:])
            wga_t = pp.tile([D, 264], f32r)
            nc.sync.dma_start(wga_t[:], WGA[:])
            w0_t = pp.tile([D, D], f32r)
            nc.sync.dma_start(w0_t[:], W0[:])
            wgn_t = pp.tile([D, D], f32r)
            nc.sync.dma_start(wgn_t[:], WGN[:])
            wot_t = pp.tile([D, D], f32r)
            nc.sync.dma_start(wot_t[:], WOT[:])
            abias_t = pp.tile([P, H], f32)
            nc.sync.dma_start(abias_t[:], ABIAS[:])
            obias_t = pp.tile([P, D], f32)
            nc.sync.dma_start(obias_t[:], OBIAS[:])
            idxa_t = pp.tile([P, W * sw16], i16)
            nc.sync.dma_start(idxa_t[:], IDX_A[:])
            idxb_t = pp.tile([P, W * sw16], i16)
            nc.sync.dma_start(idxb_t[:], IDX_B[:])
            cnt_t = pp.tile([1, W], i32)
            nc.sync.dma_start(cnt_t[:], CNT_B[:])
            trel_t = pp.tile([P, W * B], f32)
            nc.sync.dma_start(trel_t[:], TGT_REL[:])

            a_table = pp.tile([P, W * 264], f32r)
            nc.gpsimd.memset(a_table[:].bitcast(f32), 0.0)
            agg_table = pp.tile([P, W * 136], f32)

            # ---- Phase D: per-target table  A_La = Xc@[Wu|0|Wu aT] + Gc@[Wge|0|Wge aT] + [0|0|ab]
            for w in range(W):
                xcw = wp.tile([P, D], f32r, tag="xcw")
                nc.sync.dma_start(xcw[:], XC[w * P:(w + 1) * P, :])
                gcw = wp.tile([P, D], f32r, tag="gcw")
                nc.sync.dma_start(gcw[:], GC[w * P:(w + 1) * P, :])
                pt1 = ptp.tile([P, P], f32r, tag="pt")
                nc.tensor.transpose(out=pt1[:], in_=xcw[:], identity=ident[:])
                xt = wp.tile([P, P], f32r, tag="xt")
                nc.scalar.activation(xt[:], pt1[:], ACT.Copy)
                pt2 = ptp.tile([P, P], f32r, tag="pt")
                nc.tensor.transpose(out=pt2[:], in_=gcw[:], identity=ident[:])
                gt = wp.tile([P, P], f32r, tag="gt")
                nc.scalar.activation(gt[:], pt2[:], ACT.Copy)
                pa = pmp.tile([P, 264], f32, tag="pm")
                nc.tensor.matmul(out=pa[:], lhsT=xt[:], rhs=wua_t[:], start=True, stop=False)
                nc.tensor.matmul(out=pa[:], lhsT=gt[:], rhs=wga_t[:], start=False, stop=True)
                nc.vector.tensor_copy(a_table[:, w * 264:w * 264 + 128], pa[:, 0:128])
                nc.vector.tensor_add(
                    out=a_table[:, w * 264 + 256:w * 264 + 264],
                    in0=pa[:, 256:264], in1=abias_t[:],
                )

            # ---- Phase E: edge stream
            GMAX = 1024  # dma_gather crashes above ~1024 rows per call
            sub = [(s, min(s + GMAX, slotsw)) for s in range(0, slotsw, GMAX)]
            with nc.gpsimd.register("cntb") as rcnt:
                gx_tiles = {}
                qctr = [0]

                def nextq():
                    qctr[0] = (qctr[0] + 1) % 4
                    return qctr[0]

                def emit_g2(w):
                    gx = gxp.tile([P, B * P], f32r, tag="gx")
                    gx_tiles[w] = gx
                    for (lo, hi) in sub:
                        nc.gpsimd.dma_gather(
                            out_ap=gx[:, (lo // P) * D:(hi // P) * D].rearrange("p (b e) -> p b e", e=P),
                            in_ap=X_LO[:],
                            idxs_ap=idxa_t[:, w * sw16 + lo // 16:w * sw16 + hi // 16],
                            num_idxs=hi - lo, num_idxs_reg=hi - lo,
                            elem_size=D, queue_num=nextq(),
                        )

                def emit_g1(w):
                    # B-chunk overlay: real indices occupy a prefix of <= GMAX
                    # slots (host asserts cnt_b <= GMAX), so one sub-call.
                    gx = gx_tiles[w]
                    nc.gpsimd.reg_load(rcnt, cnt_t[0:1, w:w + 1])
                    nc.gpsimd.dma_gather(
                        out_ap=gx[:, 0:(GMAX // P) * D].rearrange("p (b e) -> p b e", e=P),
                        in_ap=X_HI[:],
                        idxs_ap=idxb_t[:, w * sw16:w * sw16 + GMAX // 16],
                        num_idxs=GMAX, num_idxs_reg=rcnt,
                        elem_size=D, queue_num=nextq(),
                    )

                emit_g2(0)
                for w in range(W):
                    if w + 1 < W:
                        emit_g2(w + 1)
                    emit_g1(w)
                    gx = gx_tiles.pop(w)
                    pagg = pap.tile([P, 256], f32, tag="pagg")
                    for b in range(B):
                        blk = w * B + b
                        eft_b = efp.tile([P, P], f32r, tag="ef")
                        nc.sync.dma_start(eft_b[:], EFT[blk * P:(blk + 1) * P, :])
                        ptx = ptp.tile([P, P], f32r, tag="pt")
                        nc.tensor.transpose(out=ptx[:], in_=gx[:, b * P:(b + 1) * P], identity=ident[:])
                        xsT = wp.tile([P, P], f32r, tag="xsT")
                        nc.scalar.activation(xsT[:], ptx[:], ACT.Copy)
                        pm = pmp.tile([P, 264], f32, tag="pm")
                        nc.tensor.matmul(out=pm[:], lhsT=xsT[:], rhs=wcx_t[:], start=True, stop=False)
                        nc.tensor.matmul(out=pm[:], lhsT=eft_b[:], rhs=wce_t[:], start=False, stop=False)
                        M = wp.tile([P, P], f32r, tag="M")
                        nc.vector.tensor_scalar(
                            out=M[:], in0=iota_t[:], scalar1=trel_t[:, blk:blk + 1],
                            scalar2=None, op0=ALU.is_equal,
                        )
                        ptm = ptp.tile([P, P], f32r, tag="pt")
                        nc.tensor.transpose(out=ptm[:], in_=M[:], identity=ident[:])
                        mT = wp.tile([P, P], f32r, tag="mT")
                        nc.vector.tensor_copy(mT[:], ptm[:])
                        nc.tensor.matmul(out=pm[:], lhsT=mT[:], rhs=a_table[:, w * 264:(w + 1) * 264], start=False, stop=True)
                        # outputs of this block
                        nef_s = op.tile([P, D], f32, tag="nef")
                        nc.vector.tensor_copy(nef_s[:], pm[:, 0:128])
                        nc.sync.dma_start(NEF[blk * P:(blk + 1) * P, :], nef_s[:])
                        # exp(leaky_relu(x)) == max(exp(x), exp(0.2*x))
                        # (HW Lrelu does not honor alpha, so avoid it)
                        e1 = sp.tile([P, H], f32, tag="e1")
                        nc.scalar.activation(e1[:], pm[:, 256:264], ACT.Exp)
                        e2 = sp.tile([P, H], f32, tag="e2")
                        nc.scalar.activation(e2[:], pm[:, 256:264], ACT.Exp, scale=NEG_SLOPE)
                        valw = wp.tile([P, 256], f32r, tag="valw")
                        nc.vector.tensor_tensor(out=valw[:, 128:136], in0=e1[:], in1=e2[:], op=ALU.max)
                        wexp_b = bass.AP(
                            valw[:].tensor, valw[:, 128:136].offset,
                            [valw[:].ap[0], [1, H], [0, HD]],
                        )
                        nc.vector.tensor_tensor(
                            out=valw[:, 0:128].rearrange("p (h j) -> p h j", j=HD),
                            in0=pm[:, 128:256].rearrange("p (h j) -> p h j", j=HD),
                            in1=wexp_b, op=ALU.mult,
                        )
                        nc.tensor.matmul(
                            out=pagg[:], lhsT=M[:], rhs=valw[:],
                            start=(b == 0), stop=(b == B - 1),
                        )
                    nc.vector.tensor_copy(agg_table[:, w * 136:(w + 1) * 136], pagg[:, 0:136])

            nc.sync.dma_start(AGGD[:], agg_table[:])
            # ---- Phase F: normalize + final projection
            for w in range(W):
                dn = sp.tile([P, H], f32, tag="dn")
                nc.vector.tensor_scalar(
                    out=dn[:], in0=agg_table[:, w * 136 + 128:w * 136 + 136],
                    scalar1=1e-10, scalar2=None, op0=ALU.add,
                )
                rec = sp.tile([P, H], f32, tag="rec")
                nc.vector.reciprocal(rec[:], dn[:])
                aggn = wp.tile([P, D], f32, tag="aggn")
                rec_b = bass.AP(rec[:].tensor, rec[:].offset, [rec[:].ap[0], [1, H], [0, HD]])
                nc.vector.tensor_tensor(
                    out=aggn[:].rearrange("p (h j) -> p h j", j=HD),
                    in0=agg_table[:, w * 136:w * 136 + 128].rearrange("p (h j) -> p h j", j=HD),
                    in1=rec_b, op=ALU.mult,
                )
                pta = ptp.tile([P, P], f32, tag="pt")
                nc.tensor.transpose(out=pta[:], in_=aggn[:], identity=ident_f[:])
                aggnT = wp.tile([P, P], f32, tag="aggnT")
                nc.vector.tensor_copy(aggnT[:], pta[:])
                xcw = wp.tile([P, D], f32r, tag="xcw")
                nc.sync.dma_start(xcw[:], XC[w * P:(w + 1) * P, :])
                gcw = wp.tile([P, D], f32r, tag="gcw")
                nc.sync.dma_start(gcw[:], GC[w * P:(w + 1) * P, :])
                pt1 = ptp.tile([P, P], f32r, tag="pt")
                nc.tensor.transpose(out=pt1[:], in_=xcw[:], identity=ident[:])
                xt = wp.tile([P, P], f32r, tag="xt")
                nc.scalar.activation(xt[:], pt1[:], ACT.Copy)
                pt2 = ptp.tile([P, P], f32r, tag="pt")
                nc.tensor.transpose(out=pt2[:], in_=gcw[:], identity=ident[:])
                gt = wp.tile([P, P], f32r, tag="gt")
                nc.scalar.activation(gt[:], pt2[:], ACT.Copy)
                pnh = pmp.tile([P, P], f32, tag="pm")
                nc.tensor.matmul(out=pnh[:], lhsT=w0_t[:], rhs=xt[:], start=True, stop=False)
                nc.tensor.matmul(out=pnh[:], lhsT=wgn_t[:], rhs=gt[:], start=False, stop=True)
                nhT = wp.tile([P, P], f32r, tag="nhT")
                nc.vector.tensor_add(out=nhT[:], in0=pnh[:], in1=aggnT[:])
                pf = pmp.tile([P, P], f32, tag="pm")
                nc.tensor.matmul(out=pf[:], lhsT=nhT[:], rhs=wot_t[:], start=True, stop=True)
                outf = op.tile([P, D], f32, tag="outf")
                nc.vector.tensor_add(out=outf[:], in0=pf[:], in1=obias_t[:])
                nc.sync.dma_start(NEWF[w * P:(w + 1) * P, :], outf[:])

    nc.finalize()
    return nc


_CACHED = {}


def _get_nc(meta):
    key = (meta["W"], meta["B"], meta["n_hi"])
    if key not in _CACHED:
        _CACHED[key] = _build_nc(meta)
    return _CACHED[key]


def kernel(**inputs):
    per_core, assemble, meta = _host_prep(inputs)
    nc = _get_nc(meta)
    trace = False
    res = run_bass_kernel_spmd(nc, per_core, core_ids=list(range(N_CORES)), trace=trace)
    N, E, NPC = meta["N"], meta["E"], meta["NPC"]
    new_final = np.empty((N, D), np.float32)
    nef_full = np.empty((E, D), np.float32)
    for c in range(N_CORES):
        r = res.results[c]
        a = assemble[c]
        new_final[a["n0"]:a["n0"] + NPC] = r["newf"][:NPC]
        so = a["slot_orig"]
        valid = so >= 0
        nef_full[so[valid]] = r["nef"][valid]
    kernel.last_exec_time_ns = res.exec_time_ns
    return new_final, nef_full
